# revision 1
# baseline (speedup 1.0000x reference)
"""Cost-volume (correlation) kernel for Trainium2, 8 NeuronCores.

cost[b, dh*9+dw, h, w] = sum_c feat1[b,c,h,w] * feat2[b,c,h+dh-4,w+dw-4]
with zero padding outside image bounds.

Strategy (data-parallel over batch, one batch element per core):
  - Host pre-pads feat2 and lays out fp16 "shingled" per-patch inputs:
    8x16 output patches, each with its 16x24 feat2 window contiguous.
  - TensorE computes, per patch, the full outer-product block
    S[p, q] = sum_c f1[c, p] * f2win[c, q]  ([128, 384] in PSUM) whose
    generalized diagonals contain the 81 correlation outputs.
  - The block is staged to an HBM scratch buffer with a *sheared* per-row
    placement (row p at offset 448*p - (y*24+x)), which makes the needed
    band elements line up at identical offsets across partitions.
  - A single strided re-read brings back [128, 216] per patch; a strided
    engine copy compacts the 81 band values; TensorE transposes to put the
    81 offsets on partitions; results accumulate into [81, 8, 320] strips
    written to the output with large contiguous descriptors.
"""

import os
import sys

import numpy as np

for _p in ("/opt/trn_rl_repo", os.path.expanduser("~/.axon_site/_ro/trn_rl_repo")):
    if os.path.isdir(_p) and _p not in sys.path:
        sys.path.insert(0, _p)

C, H, W = 96, 160, 320
B = 8
MH = MW = 4
PH, PW = 8, 16          # output patch
RH, RW = 16, 24         # feat2 window per patch
NPX = W // PW           # 20 patches per band
NBAND = H // PH         # 20 bands
NPATCH = NBAND * NPX    # 400
N = RH * RW             # 384
D = 81
STG = 448 * 128 + 16    # staged elements per patch

_cache = {}


def _build():
    import concourse.bass as bass
    import concourse.tile as tile
    from concourse import bacc, mybir

    f32, f16 = mybir.dt.float32, mybir.dt.float16
    nc = bacc.Bacc("TRN2", target_bir_lowering=False, debug=False, num_devices=8)
    f1sh = nc.dram_tensor("f1sh", (NPATCH, C, PH * PW), f16, kind="ExternalInput")
    f2sh = nc.dram_tensor("f2sh", (NPATCH, C, N), f16, kind="ExternalInput")
    ident = nc.dram_tensor("ident", (128, 128), f32, kind="ExternalInput")
    out = nc.dram_tensor("out", (D, H, W), f32, kind="ExternalOutput")
    NSTG = 3
    stages = [nc.dram_tensor(f"stage{i}", (NPX * STG,), f16, kind="Internal")
              for i in range(NSTG)]

    with tile.TileContext(nc) as tc:
        with (
            tc.tile_pool(name="inp", bufs=3) as inp,
            tc.tile_pool(name="work", bufs=3) as work,
            tc.tile_pool(name="strip", bufs=3) as strip_pool,
            tc.tile_pool(name="psA", bufs=4, space="PSUM") as psA,
            tc.tile_pool(name="psB", bufs=4, space="PSUM") as psB,
            tc.tile_pool(name="const", bufs=1) as constp,
        ):
            idt = constp.tile([128, 128], f32)
            nc.sync.dma_start(idt[:], ident[:])
            for band in range(NBAND):
                p0 = band * NPX
                f1b = inp.tile([C, NPX * PH * PW], f16, tag="f1b")
                nc.sync.dma_start(
                    f1b[:].rearrange("c (pi x) -> c pi x", x=PH * PW),
                    f1sh[p0:p0 + NPX].transpose([1, 0, 2]))
                f2b = inp.tile([C, NPX * N], f16, tag="f2b")
                nc.sync.dma_start(
                    f2b[:].rearrange("c (pi x) -> c pi x", x=N),
                    f2sh[p0:p0 + NPX].transpose([1, 0, 2]))

                blk = work.tile([128, NPX * N], f16, tag="blk")
                for px in range(NPX):
                    acc = psA.tile([128, N], f32, tag="acc")
                    nc.tensor.matmul(acc[:], f1b[:, px * 128:(px + 1) * 128],
                                     f2b[:, px * N:(px + 1) * N],
                                     start=True, stop=True)
                    if px % 2 == 0:
                        nc.vector.tensor_copy(blk[:, px * N:(px + 1) * N], acc[:])
                    else:
                        nc.scalar.copy(blk[:, px * N:(px + 1) * N], acc[:])

                stg = stages[band % NSTG]
                for y in range(8):
                    dst = bass.AP(stg, y * (16 * 448 - 24),
                                  [[447, 16], [STG, NPX], [1, N]])
                    nc.sync.dma_start(
                        dst,
                        blk[y * 16:(y + 1) * 16, :].rearrange(
                            "p (pi q) -> p pi q", q=N))

                shb = work.tile([128, NPX * 216], f16, tag="shb")
                src = bass.AP(stg, 0, [[448, 128], [STG, NPX], [1, 216]])
                nc.sync.dma_start(
                    shb[:].rearrange("p (pi i) -> p pi i", i=216), src)

                cmpb = work.tile([128, NPX * D], f32, tag="cmpb")
                bv = shb[:].rearrange(
                    "p (pi a b) -> p pi a b", a=9, b=24)[:, :, :, 0:9]
                nc.vector.tensor_copy(
                    cmpb[:].rearrange("p (pi a b) -> p pi a b", a=9, b=9), bv)

                strip = strip_pool.tile([D, PH * W], f32, tag="strip")
                strip_v = strip[:].rearrange("d (y w) -> d y w", w=W)
                for px in range(NPX):
                    tp = psB.tile([D, 128], f32, tag="tp")
                    nc.tensor.transpose(tp[:], cmpb[:, px * D:(px + 1) * D], idt[:])
                    tv = tp[:].rearrange("d (y x) -> d y x", x=PW)
                    if px % 2 == 0:
                        nc.scalar.copy(strip_v[:, :, px * PW:(px + 1) * PW], tv)
                    else:
                        nc.vector.tensor_copy(
                            strip_v[:, :, px * PW:(px + 1) * PW], tv)

                nc.sync.dma_start(
                    out[:, band * PH:(band + 1) * PH, :].rearrange(
                        "d y w -> d (y w)"),
                    strip[:])
    nc.compile()
    return nc


def _host_prep(feat1_b, feat2_b):
    """feat1_b, feat2_b: [C, H, W] float32 -> shingled fp16 patch inputs."""
    f1 = feat1_b.astype(np.float16)
    f2p = np.pad(feat2_b, ((0, 0), (MH, MH), (MW, MW))).astype(np.float16)
    f1sh = (f1.reshape(C, NBAND, PH, NPX, PW)
            .transpose(1, 3, 0, 2, 4).reshape(NPATCH, C, PH * PW))
    s = f2p.strides
    win = np.lib.stride_tricks.as_strided(
        f2p, shape=(NBAND, NPX, C, RH, RW),
        strides=(s[1] * PH, s[2] * PW, s[0], s[1], s[2]))
    f2sh = win.reshape(NPATCH, C, N).copy()
    return np.ascontiguousarray(f1sh), f2sh


def kernel(feat1, feat2, max_offset_h, max_offset_w):
    from concourse.bass_utils import run_bass_kernel_spmd

    assert int(max_offset_h) == MH and int(max_offset_w) == MW
    feat1 = np.asarray(feat1, dtype=np.float32)
    feat2 = np.asarray(feat2, dtype=np.float32)
    assert feat1.shape == (B, C, H, W) and feat2.shape == (B, C, H, W)

    if "nc" not in _cache:
        _cache["nc"] = _build()
    nc = _cache["nc"]

    ident = np.eye(128, dtype=np.float32)
    in_maps = []
    for b in range(B):
        f1sh, f2sh = _host_prep(feat1[b], feat2[b])
        in_maps.append({"f1sh": f1sh, "f2sh": f2sh, "ident": ident})

    res = run_bass_kernel_spmd(nc, in_maps, core_ids=list(range(8)))
    return np.stack([res.results[b]["out"] for b in range(B)], axis=0)

